# revision 10
# baseline (speedup 1.0000x reference)
"""Trainium2 Bass kernel for CVectorQuantiser (VQ codebook lookup).

Problem: z [16,256,32,32] f32, weight [8192,256] f32 (l2-normalized rows).
  zc  = l2norm(z tokens)              # 16384 tokens of dim 256
  d   = 2*zc@w.T - |w_k|^2 (+const)   # [16384, 8192]
  idx = argmax_k d                    # first-max, like jnp.argmax
  z_q = w[idx] (straight-through: zc + (w[idx]-zc)), loss = beta*mean((w[idx]-zc)^2)

Sharding: data-parallel over tokens. 8 cores x 2048 tokens (2 'b' images each),
codebook replicated. Host only concatenates shards and sums 8 loss partials.

Per-core dataflow (all fp32):
  - one-time: PE-transpose weight -> wT [256d x 8192k] in SBUF; wsq row
    broadcast into [128, 8192] via PE ones-trick.
  - normalize: sumsq via PE ones-reduce, ACT sqrt(0.25*ss), DVE reciprocal
    -> inv2 = 2/norm; PE-broadcast row; zc2 = z*inv2; zc = 0.5*zc2.
  - scores: 16 token-tiles x 4 quarters: PE matmul (lhsT=zc2 tile [128d,128t],
    rhs=wT [128d,512k], 2 d-chunks accumulate) -> PSUM [128,2048];
    DVE tensor_tensor_reduce: scores_sbuf = psum - wsq_bcast, accum=quarter max.
  - argmax: per tile max8(quarter maxes) -> max_index over [128, 8192] row.
  - gather: idx -> int16 wrapped layout (DRAM bounce), gpsimd ap_gather from wT.
  - outputs: diff = wq - zc; loss partial = sum(diff^2); z_q = zc + diff.
"""

import sys
import numpy as np

sys.path.insert(0, "/opt/trn_rl_repo")

B_FULL, C, H, W = 16, 256, 32, 32
HW = H * W                      # 1024
N_CORES = 8
B_LOC = B_FULL // N_CORES       # 2 images per core
T = B_LOC * HW                  # 2048 tokens per core
K = 8192                        # codebook size
D = 256                         # code dim
N_TILES = T // 128              # 16 token tiles
N_QUART = 4                     # quarters of K per tile (2048 codes each)
QK = K // N_QUART               # 2048
BETA = 0.25

_CACHE = {}


def _build_program():
    import concourse.bacc as bacc
    import concourse.tile as tile
    import concourse.mybir as mybir
    from concourse import bass

    f32 = mybir.dt.float32
    i32 = mybir.dt.int32
    u32 = mybir.dt.uint32
    i16 = mybir.dt.int16
    X = mybir.AxisListType.X
    Alu = mybir.AluOpType
    Act = mybir.ActivationFunctionType

    nc = bacc.Bacc("TRN2", target_bir_lowering=False, debug=False)

    z_in = nc.dram_tensor("z", [B_LOC, C, HW], f32, kind="ExternalInput").ap()
    w_in = nc.dram_tensor("weight", [K, D], f32, kind="ExternalInput").ap()
    id_in = nc.dram_tensor("id128", [128, 128], f32, kind="ExternalInput").ap()
    zq_out = nc.dram_tensor("z_q", [B_LOC, C, HW], f32, kind="ExternalOutput").ap()
    idx_out = nc.dram_tensor("idx", [T], i32, kind="ExternalOutput").ap()
    loss_out = nc.dram_tensor("loss_sum", [1, 1], f32, kind="ExternalOutput").ap()
    idx_bounce = nc.dram_tensor("idx_bounce", [T], i16).ap()

    with tile.TileContext(nc) as tc:
        import contextlib
        with contextlib.ExitStack() as ctx:
            # ---------------- pools ----------------
            big = ctx.enter_context(tc.tile_pool(name="big", bufs=1))
            sc_pool = ctx.enter_context(tc.tile_pool(name="scores", bufs=1))
            ztmp_pool = ctx.enter_context(tc.tile_pool(name="ztmp", bufs=1))
            wtmp_pool = ctx.enter_context(tc.tile_pool(name="wtmp", bufs=3))
            small = ctx.enter_context(tc.tile_pool(name="small", bufs=1))
            tiny = ctx.enter_context(tc.tile_pool(name="tiny", bufs=4))
            psum = ctx.enter_context(tc.tile_pool(name="ps", bufs=2, space="PSUM"))

            # ---------------- fixed tiles ----------------
            wT = big.tile([128, 2 * K], f32)          # [d%128, dc*K + k]
            wsq_bc = big.tile([128, K], f32)          # wsq broadcast across parts
            zc2 = big.tile([128, 2 * T], f32)         # 2*zc   [dc*T + t]
            wq = big.tile([128, 2 * T], f32)          # gathered codes (then diff)
            idx_all = big.tile([128, N_TILES], u32)
            idx_all16 = big.tile([128, N_TILES], i16)
            idxs_w = big.tile([128, 128], i16)        # wrapped idx for gathers
            id128 = big.tile([128, 128], f32)
            ones_col = big.tile([128, 1], f32)
            ones_row = big.tile([1, 128], f32)
            lpart1 = big.tile([128, 1], f32)
            loss_sb = big.tile([1, 1], f32)

            scores = sc_pool.tile([128, K], f32)      # one token-tile's scores

            nc.vector.memset(ones_col[:], 1.0)
            nc.vector.memset(ones_row[:], 1.0)
            nc.sync.dma_start(out=id128[:], in_=id_in[:])

            # ---------------- weight transpose: wT[d, k] ----------------
            for kb in range(K // 128):
                wtmp = wtmp_pool.tile([128, D], f32, tag="wtmp")
                nc.sync.dma_start(out=wtmp[:], in_=w_in[kb * 128:(kb + 1) * 128, :])
                for dc in range(2):
                    pt = psum.tile([128, QK], f32, tag="ps")
                    nc.tensor.transpose(
                        pt[:, 0:128], wtmp[:, dc * 128:(dc + 1) * 128], id128[:]
                    )
                    nc.scalar.activation(
                        wT[:, dc * K + kb * 128: dc * K + (kb + 1) * 128],
                        pt[:, 0:128], Act.Copy,
                    )

            # ---------------- wsq row + broadcast ----------------
            # square wT into scores scratch, PE ones-reduce -> [1, 2048] psum
            # per strip of 2048 codes, then PE-broadcast across partitions.
            for strip in range(4):
                sq = scores  # scratch [128, K]; use strip slice
                for dc in range(2):
                    nc.scalar.activation(
                        sq[:, dc * 2048: dc * 2048 + 2048],
                        wT[:, dc * K + strip * 2048: dc * K + strip * 2048 + 2048],
                        Act.Square,
                    )
                ps_strip = psum.tile([128, QK], f32, tag="ps")
                for dc in range(2):
                    for nb in range(4):
                        nc.tensor.matmul(
                            ps_strip[0:1, nb * 512:(nb + 1) * 512],
                            ones_col[:],
                            sq[:, dc * 2048 + nb * 512: dc * 2048 + (nb + 1) * 512],
                            start=(dc == 0), stop=(dc == 1),
                        )
                # broadcast rows into wsq_bc via PE: out[128, 512] = ones_rowT @ wsq_row
                # need wsq strip in SBUF first
                wsq_row = small.tile([1, 2048], f32, tag="wsqrow")
                nc.scalar.activation(wsq_row[:], ps_strip[0:1, 0:2048], Act.Copy)
                ps_bc = psum.tile([128, QK], f32, tag="ps")
                for nb in range(4):
                    nc.tensor.matmul(
                        ps_bc[:, nb * 512:(nb + 1) * 512],
                        ones_row[:],
                        wsq_row[:, nb * 512:(nb + 1) * 512],
                        start=True, stop=True,
                    )
                nc.scalar.activation(
                    wsq_bc[:, strip * 2048:(strip + 1) * 2048], ps_bc[:], Act.Copy
                )

            # ---------------- load z + normalize ----------------
            # z layout in SBUF: zc2[:, dc*T + b*HW + hw]
            zraw = big.tile([128, 2 * T], f32)
            for b in range(B_LOC):
                for dc in range(2):
                    nc.sync.dma_start(
                        out=zraw[:, dc * T + b * HW: dc * T + (b + 1) * HW],
                        in_=z_in[b, dc * 128:(dc + 1) * 128, :],
                    )
            for b in range(B_LOC):
                # sumsq via PE ones-reduce -> psum [1, HW]
                zsq = ztmp_pool.tile([128, HW], f32, tag="zsq")
                ps_ss = psum.tile([128, QK], f32, tag="ps")
                for dc in range(2):
                    nc.vector.tensor_tensor(
                        zsq[:],
                        zraw[:, dc * T + b * HW: dc * T + (b + 1) * HW],
                        zraw[:, dc * T + b * HW: dc * T + (b + 1) * HW],
                        Alu.mult,
                    )
                    for nb in range(2):
                        nc.tensor.matmul(
                            ps_ss[0:1, nb * 512:(nb + 1) * 512],
                            ones_col[:],
                            zsq[:, nb * 512:(nb + 1) * 512],
                            start=(dc == 0), stop=(dc == 1),
                        )
                # snorm = sqrt(0.25*ss) = norm/2 ; inv2 = 1/snorm = 2/norm
                snorm_row = small.tile([1, HW], f32, tag="snorm")
                inv2_row = small.tile([1, HW], f32, tag="inv2")
                nc.scalar.activation(
                    snorm_row[:], ps_ss[0:1, 0:HW], Act.Sqrt, scale=0.25,
                )
                nc.vector.reciprocal(inv2_row[:], snorm_row[:])
                # broadcast inv2 into [128, HW] psum, then zc2 = zraw * bcast
                ps_bc2 = psum.tile([128, QK], f32, tag="ps")
                for nb in range(2):
                    nc.tensor.matmul(
                        ps_bc2[:, nb * 512:(nb + 1) * 512],
                        ones_row[:],
                        inv2_row[:, nb * 512:(nb + 1) * 512],
                        start=True, stop=True,
                    )
                for dc in range(2):
                    nc.vector.tensor_tensor(
                        zc2[:, dc * T + b * HW: dc * T + (b + 1) * HW],
                        zraw[:, dc * T + b * HW: dc * T + (b + 1) * HW],
                        ps_bc2[:, 0:HW],
                        Alu.mult,
                    )


            # ---------------- main: scores + argmax ----------------
            for t in range(N_TILES):
                for q in range(N_QUART):
                    ps_q = psum.tile([128, QK], f32, tag="ps")
                    for dc in range(2):
                        lhsT = zc2[:, dc * T + t * 128: dc * T + (t + 1) * 128]
                        for nb in range(4):
                            nc.tensor.matmul(
                                ps_q[:, nb * 512:(nb + 1) * 512],
                                lhsT,
                                wT[:, dc * K + q * QK + nb * 512:
                                   dc * K + q * QK + (nb + 1) * 512],
                                start=(dc == 0), stop=(dc == 1),
                            )
                    # corrected scores = 2*cos - wsq  (PSUM -> SBUF)
                    nc.vector.tensor_tensor(
                        scores[:, q * QK:(q + 1) * QK],
                        ps_q[:],
                        wsq_bc[:, q * QK:(q + 1) * QK],
                        Alu.subtract,
                    )
                cmax = tiny.tile([128, 16], f32, tag="cmax")
                nc.vector.tensor_reduce(
                    cmax[:], scores.rearrange("p (c g) -> p c g", c=16),
                    axis=X, op=Alu.max,
                )
                m8 = tiny.tile([128, 8], f32, tag="m8")
                i8 = tiny.tile([128, 8], u32, tag="i8")
                nc.vector.max(m8[:], cmax[:])
                nc.vector.max_index(i8[:], m8[:], scores[:])
                nc.vector.tensor_copy(idx_all[:, t:t + 1], i8[:, 0:1])

            # ---------------- idx -> wrapped int16 layout ----------------
            nc.vector.tensor_copy(idx_all16[:], idx_all[:])
            nc.sync.dma_start(
                out=idx_bounce.rearrange("(j p) -> p j", p=128), in_=idx_all16[:]
            )
            src_w = idx_bounce.rearrange("(s q) -> q s", q=16)
            for g in range(8):
                nc.sync.dma_start(out=idxs_w[16 * g:16 * (g + 1), :], in_=src_w)
            # idx output (int32)
            nc.sync.dma_start(
                out=idx_out.rearrange("(j p) -> p j", p=128),
                in_=idx_all[:].bitcast(i32),
            )

            # ---------------- gather codes: wq[d, t] ----------------
            for dc in range(2):
                nc.gpsimd.ap_gather(
                    wq[:, dc * T:(dc + 1) * T],
                    wT[:, dc * K:(dc + 1) * K],
                    idxs_w[:],
                    channels=128, num_elems=K, d=1, num_idxs=T,
                )

            # ---------------- diff, loss, z_q ----------------
            # zc2 is dead after the last matmul: halve it in place -> zc
            zc = zc2
            nc.vector.tensor_scalar_mul(zc[:], zc2[:], 0.5)
            diff = wq  # in-place: diff = wq - zc
            nc.vector.tensor_tensor(diff[:], wq[:], zc[:], Alu.subtract)
            # loss partial: sum(diff^2) per partition via ACT Square + accum
            nc.scalar.activation(
                scores[:, 0:2 * T], diff[:], Act.Square, accum_out=lpart1[:],
            )
            ps_l = psum.tile([128, QK], f32, tag="ps")
            nc.tensor.matmul(ps_l[0:1, 0:1], lpart1[:], ones_col[:],
                             start=True, stop=True)
            nc.scalar.activation(loss_sb[:], ps_l[0:1, 0:1], Act.Copy)
            nc.sync.dma_start(out=loss_out[:], in_=loss_sb[:])

            zq_sb = zraw  # reuse
            nc.vector.tensor_tensor(zq_sb[:], zc[:], diff[:], Alu.add)
            for b in range(B_LOC):
                for dc in range(2):
                    nc.sync.dma_start(
                        out=zq_out[b, dc * 128:(dc + 1) * 128, :],
                        in_=zq_sb[:, dc * T + b * HW: dc * T + (b + 1) * HW],
                    )

    nc.compile()
    return nc


def _get_program():
    if "nc" not in _CACHE:
        _CACHE["nc"] = _build_program()
    return _CACHE["nc"]


def kernel(z: np.ndarray, weight: np.ndarray):
    from concourse.bass_utils import run_bass_kernel_spmd

    nc = _get_program()
    z = np.ascontiguousarray(z, dtype=np.float32).reshape(B_FULL, C, HW)
    weight = np.ascontiguousarray(weight, dtype=np.float32)
    id128 = np.eye(128, dtype=np.float32)

    in_maps = []
    for c in range(N_CORES):
        in_maps.append({
            "z": z[c * B_LOC:(c + 1) * B_LOC],
            "weight": weight,
            "id128": id128,
        })
    res = run_bass_kernel_spmd(nc, in_maps, list(range(N_CORES)))

    zq = np.concatenate(
        [res.results[c]["z_q"].reshape(B_LOC, C, H, W) for c in range(N_CORES)],
        axis=0,
    )
    idx = np.concatenate([res.results[c]["idx"] for c in range(N_CORES)])
    total = np.sum([np.float64(res.results[c]["loss_sum"][0, 0])
                    for c in range(N_CORES)])
    loss = np.float32(BETA * total / (B_FULL * HW * C))
    return zq, loss, idx.astype(np.int32)
